# revision 32
# baseline (speedup 1.0000x reference)
"""Multi-head attention (B=2, S=2048, D=1024, H=16) on 8 trn2 NeuronCores.

Sharding: 2-way batch x 4-way head-group tensor parallel. Core c handles
batch c//4 and heads 4*(c%4) .. 4*(c%4)+3 (a 256-wide feature slice of the
q/k/v projections, and the matching row-slice of the out projection). Each
core emits a full-size [2048, 1024] partial of the output; the host sums the
4 partials per batch and adds the output bias.

On-device dataflow (per core):
  phase A: PE-transpose x chunks (f32r, 1.5 cyc/row) into [f, t] layout,
           evicting PSUM in groups of 4 tiles alternately on DVE and ScalarE;
           project to QT/KT [dq, t] (feature-major) and V [t, dv]
           (token-major). V gets a 64-wide block of ones columns appended so
           the attn.V matmul also produces the softmax denominator
           replicated on psum partitions 64..127.
  phase B: per (q-chunk, head): scoresT[k, q] = KT_h.T @ QT_h on PE (f32r),
           exp(0.125 * s) on ScalarE in [128, 1024]-wide ACTIVATEs (scores
           are small, so no max-subtraction is needed), then
           outT'[128, q] = sum_k V''_h.T @ P. Rows 64..127 are the softmax
           denominator; normalize rows 0..63 via reciprocal_approx_fast +
           multiply on DVE.
           After all 4 heads of a q-chunk: out-projection matmuls for those
           4 token tiles (keeps PE fed while ACT runs exp for the next
           chunk).
"""

import ml_dtypes
import numpy as np

import concourse.bacc as bacc
import concourse.bass as bass
import concourse.mybir as mybir
import concourse.tile as tile
from concourse.bass_interp import get_hw_module
from concourse.bass_utils import run_bass_kernel_spmd
from concourse.masks import make_identity

# problem constants (hardcoded; must match the reference)
B = 2
S = 2048
D = 1024
NH = 16
DH = 64
SCALE = DH ** -0.5

# sharding
N_CORES = 8
HG = 4                # heads per core
F = HG * DH           # 256 projected features per core
CH = 512              # token chunk
NCH = S // CH         # 4 chunks
P = 128
FT = D // P           # 8 feature tiles
MT = F // P           # 2 projected-feature tiles
KT = S // P           # 16 key-token tiles

f32 = mybir.dt.float32
f32r = mybir.dt.float32r
bf16 = mybir.dt.bfloat16
EXP = mybir.ActivationFunctionType.Exp


def _emit(ctx, nc, tc, aps):
    xq, xk, xv, wqT, wkT, wvT, woT, bq2, bk2, bv1, out = aps

    consts = ctx.enter_context(tc.tile_pool(name="consts", bufs=1))
    persist = ctx.enter_context(tc.tile_pool(name="persist", bufs=1))
    # weights / biases to SBUF (q/k/v projection weights are loaded
    # per-input inside phase A to save SBUF)
    wo_sb = consts.tile([P, MT, D], f32r)
    nc.scalar.dma_start(out=wo_sb, in_=woT.rearrange("(m p) e -> p m e", p=P))
    bq_sb = consts.tile([P, MT], f32)
    bk_sb = consts.tile([P, MT], f32)
    nc.scalar.dma_start(out=bq_sb, in_=bq2)
    nc.scalar.dma_start(out=bk_sb, in_=bk2)
    bv_sb = consts.tile([P, F], f32)
    nc.scalar.dma_start(out=bv_sb, in_=bv1.unsqueeze(0).to_broadcast((P, F)))

    # persistent activations
    QT_sb = persist.tile([P, MT, NCH, CH], f32r)   # [dq%128, dq//128, qc, q]
    # KT, zero-padded to full-K contraction: variant par holds head parity
    # par's 64 feature rows, zeros in the other 64. A scores matmul then uses
    # a full [128, 128] stationary operand (K=64 descriptors run at half PE
    # rate), with the zeros annihilating the other head's QT rows.
    KTz_sb = persist.tile([P, 2, MT, NCH, CH], f32r)
    # V'' layout: [k%128, k//128, h, dv | 64 ones columns]
    V_sb = persist.tile([P, KT, HG, P], f32r)
    ones_sb = consts.tile([P, 1], f32)
    nc.vector.memset(ones_sb, 1.0)
    nc.vector.tensor_copy(
        V_sb[:, :, :, DH:P], ones_sb.to_broadcast((P, KT, HG, P - DH))
    )
    zeros_sb = consts.tile([P, 1], f32)
    nc.vector.memset(zeros_sb, 0.0)
    nc.vector.tensor_copy(
        KTz_sb[DH:P, 0], zeros_sb[DH:P].to_broadcast((DH, MT, NCH, CH))
    )
    nc.vector.tensor_copy(
        KTz_sb[0:DH, 1], zeros_sb[0:DH].to_broadcast((DH, MT, NCH, CH))
    )

    identity = consts.tile([P, P], f32r)
    id_tmp = consts.tile([P, P], f32)
    make_identity(nc, id_tmp)
    nc.vector.tensor_copy(identity, id_tmp)

    n_evict = 0
    with tc.tile_pool(name="x_stage", bufs=2) as x_stage, \
         tc.tile_pool(name="w_pool", bufs=2) as w_pool, \
         tc.tile_pool(name="xT_pool", bufs=3) as xT_pool, \
         tc.tile_pool(name="ps_tr", bufs=4, space="PSUM") as ps_tr, \
         tc.tile_pool(name="ps_proj", bufs=3, space="PSUM") as ps_proj:
        # phase A: transpose + project, per (input, chunk). K and V first so
        # attention (which needs all of K/V but only Q's first chunk) can
        # begin while Q's later chunks still project.
        for which, (x_ap, wT_ap) in enumerate(
            ((xk, wkT), (xv, wvT), (xq, wqT))
        ):
            w_sb = w_pool.tile([P, FT, F], f32r, tag="w")
            nc.scalar.dma_start(
                out=w_sb, in_=wT_ap.rearrange("(ft p) m -> p ft m", p=P)
            )
            for c in range(NCH):
                xT = xT_pool.tile([P, FT, CH], f32r, tag="xT")
                xs4 = x_stage.tile([P, CH // P, D], f32r, tag="xs")
                nc.sync.dma_start(
                    out=xs4,
                    in_=x_ap[c * CH:(c + 1) * CH, :].rearrange(
                        "(t4 p) f -> p t4 f", p=P
                    ),
                )
                for t4 in range(CH // P):
                    xs = xs4[:, t4, :]
                    for fg in range(2):  # groups of 4 feature tiles
                        ps = ps_tr.tile([P, 4, P], f32r, tag="tr")
                        for j in range(4):
                            ft = fg * 4 + j
                            nc.tensor.transpose(
                                ps[:, j, :], xs[:, ft * P:(ft + 1) * P],
                                identity,
                            )
                        dst = xT.rearrange("p (fg j) t -> p fg j t", fg=2)
                        if n_evict % 2 == 0:
                            nc.vector.tensor_copy(
                                dst[:, fg, :, t4 * P:(t4 + 1) * P], ps
                            )
                        else:
                            nc.scalar.copy(
                                dst[:, fg, :, t4 * P:(t4 + 1) * P], ps
                            )
                        n_evict += 1
                if which != 1:  # Q / K: feature-major [dq, t]
                    is_q = which == 2
                    b_sb = bq_sb if is_q else bk_sb
                    for m in range(MT):
                        ps = ps_proj.tile([P, CH], f32, tag="proj")
                        for ft in range(FT):
                            nc.tensor.matmul(
                                ps,
                                w_sb[:, ft, m * P:(m + 1) * P],
                                xT[:, ft, :],
                                start=(ft == 0),
                                stop=(ft == FT - 1),
                            )
                        if is_q:
                            nc.vector.tensor_scalar_add(
                                QT_sb[:, m, c, :], ps, b_sb[:, m:m + 1]
                            )
                        else:
                            nc.vector.tensor_scalar_add(
                                KTz_sb[0:DH, 0, m, c, :], ps[0:DH, :],
                                b_sb[0:DH, m:m + 1],
                            )
                            nc.vector.tensor_scalar_add(
                                KTz_sb[DH:P, 1, m, c, :], ps[DH:P, :],
                                b_sb[DH:P, m:m + 1],
                            )
                else:  # V: token-major [t, dv]
                    for t4 in range(CH // P):
                        ps = ps_proj.tile([P, F], f32, tag="proj")
                        for ft in range(FT):
                            nc.tensor.matmul(
                                ps,
                                xT[:, ft, t4 * P:(t4 + 1) * P],
                                w_sb[:, ft, :],
                                start=(ft == 0),
                                stop=(ft == FT - 1),
                            )
                        kt = c * (CH // P) + t4
                        nc.vector.tensor_add(
                            V_sb[:, kt, :, 0:DH],
                            ps.rearrange("p (h d) -> p h d", h=HG),
                            bv_sb.rearrange("p (h d) -> p h d", h=HG),
                        )

    with tc.tile_pool(name="ps_s", bufs=2, space="PSUM") as ps_s, \
         tc.tile_pool(name="ps_o", bufs=2, space="PSUM") as ps_o, \
         tc.tile_pool(name="ps_out", bufs=2, space="PSUM") as ps_out, \
         tc.tile_pool(name="pt_pool", bufs=1) as pt_pool, \
         tc.tile_pool(name="ot_pool", bufs=1) as ot_pool, \
         tc.tile_pool(name="o_stage", bufs=3) as o_stage, \
         tc.tile_pool(name="rc_pool", bufs=1) as rc_pool:
        OT_sb = ot_pool.tile([P, MT, NCH, CH], f32r)
        # phase B: attention per (q-chunk, head), then the chunk's out-proj
        for qc in range(NCH):
            for h in range(HG):
                mh, p0 = divmod(h, 2)
                p0 *= DH
                PT = pt_pool.tile([P, KT, CH], f32r, tag="PT")
                par = h % 2
                for kg in range(KT // 2):  # 2 k-tiles share a psum group
                    ps = ps_s.tile([P, 2, CH], f32, tag="s")
                    for j in range(2):
                        kt = kg * 2 + j
                        nc.tensor.matmul(
                            ps[:, j, :],
                            KTz_sb[:, par, mh, kt // 4,
                                   (kt % 4) * P:(kt % 4) * P + P],
                            QT_sb[:, mh, qc, :],
                            start=True,
                            stop=True,
                        )
                    nc.scalar.activation(
                        out=PT[:, kg * 2:kg * 2 + 2, :], in_=ps,
                        func=EXP, scale=SCALE,
                    )
                po = ps_o.tile([P, CH], f32, tag="o")
                for kt in range(KT):
                    nc.tensor.matmul(
                        po,
                        V_sb[:, kt, h, :],
                        PT[:, kt, :],
                        start=(kt == 0),
                        stop=(kt == KT - 1),
                    )
                rs = rc_pool.tile([DH, CH], f32, tag="rs")
                rc = rc_pool.tile([DH, CH], f32, tag="rc")
                nc.vector.tensor_copy(rs, po[DH:P, :])
                nc.vector.reciprocal_approx_fast(rc, rs)
                nc.vector.tensor_mul(
                    OT_sb[p0:p0 + DH, mh, qc, :], po[0:DH, :], rc
                )
            # out projection for this chunk's 4 token tiles
            for t4 in range(NCH):
                ob = o_stage.tile([P, D], f32, tag="ob")
                for n2 in range(D // CH):
                    ps = ps_out.tile([P, CH], f32, tag="po")
                    for m in range(MT):
                        nc.tensor.matmul(
                            ps,
                            OT_sb[:, m, qc, t4 * P:(t4 + 1) * P],
                            wo_sb[:, m, n2 * CH:(n2 + 1) * CH],
                            start=(m == 0),
                            stop=(m == MT - 1),
                        )
                    nc.vector.tensor_copy(ob[:, n2 * CH:(n2 + 1) * CH], ps)
                tt = qc * NCH + t4
                nc.scalar.dma_start(out=out[tt * P:(tt + 1) * P, :], in_=ob)


def _build():
    nc = bacc.Bacc("TRN2", target_bir_lowering=False, debug=False)
    xq = nc.dram_tensor("xq", [S, D], f32r, kind="ExternalInput").ap()
    xk = nc.dram_tensor("xk", [S, D], f32r, kind="ExternalInput").ap()
    xv = nc.dram_tensor("xv", [S, D], f32r, kind="ExternalInput").ap()
    wqT = nc.dram_tensor("wqT", [D, F], f32r, kind="ExternalInput").ap()
    wkT = nc.dram_tensor("wkT", [D, F], f32r, kind="ExternalInput").ap()
    wvT = nc.dram_tensor("wvT", [D, F], f32r, kind="ExternalInput").ap()
    woT = nc.dram_tensor("woT", [F, D], f32r, kind="ExternalInput").ap()
    bq2 = nc.dram_tensor("bq2", [P, MT], f32, kind="ExternalInput").ap()
    bk2 = nc.dram_tensor("bk2", [P, MT], f32, kind="ExternalInput").ap()
    bv1 = nc.dram_tensor("bv1", [F], f32, kind="ExternalInput").ap()
    out = nc.dram_tensor("out", [S, D], f32, kind="ExternalOutput").ap()
    from contextlib import ExitStack

    with tile.TileContext(nc) as tc, ExitStack() as ctx:
        _emit(ctx, nc, tc,
              (xq, xk, xv, wqT, wkT, wvT, woT, bq2, bk2, bv1, out))
    nc.compile()
    nc.m = get_hw_module(nc.m)
    return nc


_cached_nc = None


def _get_nc():
    global _cached_nc
    if _cached_nc is None:
        _cached_nc = _build()
    return _cached_nc


def make_in_maps(query, key, value, Wq, bq, Wk, bk, Wv, bv, Wo, bo):
    query, key, value, Wq, bq, Wk, bk, Wv, bv, Wo = (
        np.asarray(a, np.float32)
        for a in (query, key, value, Wq, bq, Wk, bk, Wv, bv, Wo)
    )
    in_maps = []
    for c in range(N_CORES):
        b, g = divmod(c, 4)
        fs = slice(g * F, (g + 1) * F)
        in_maps.append({
            "xq": np.ascontiguousarray(query[b]),
            "xk": np.ascontiguousarray(key[b]),
            "xv": np.ascontiguousarray(value[b]),
            "wqT": np.ascontiguousarray(Wq[fs].T),
            "wkT": np.ascontiguousarray(Wk[fs].T),
            "wvT": np.ascontiguousarray(Wv[fs].T),
            "woT": np.ascontiguousarray(Wo[:, fs].T),
            "bq2": np.ascontiguousarray(bq[fs].reshape(MT, P).T),
            "bk2": np.ascontiguousarray(bk[fs].reshape(MT, P).T),
            "bv1": np.ascontiguousarray(bv[fs]),
        })
    return in_maps


def combine_outputs(core_outs, bo):
    bo = np.asarray(bo, np.float32)
    out = np.empty((B, S, D), np.float32)
    for b in range(B):
        acc = core_outs[4 * b].astype(np.float32)
        for g in range(1, 4):
            acc = acc + core_outs[4 * b + g]
        out[b] = acc + bo
    return out


def kernel(query, key, value, Wq, bq, Wk, bk, Wv, bv, Wo, bo, **run_kwargs):
    nc = _get_nc()
    in_maps = make_in_maps(query, key, value, Wq, bq, Wk, bk, Wv, bv, Wo, bo)
    res = run_bass_kernel_spmd(
        nc, in_maps, core_ids=list(range(N_CORES)), **run_kwargs
    )
    out = combine_outputs([r["out"] for r in res.results], bo)
    if run_kwargs:
        kernel.last_results = res
    return out


# revision 33
# speedup vs baseline: 1.0916x; 1.0916x over previous
"""Multi-head attention (B=2, S=2048, D=1024, H=16) on 8 trn2 NeuronCores.

Sharding: 2-way batch x 4-way head-group tensor parallel. Core c handles
batch c//4 and heads 4*(c%4) .. 4*(c%4)+3 (a 256-wide feature slice of the
q/k/v projections, and the matching row-slice of the out projection). Each
core emits a full-size [2048, 1024] partial of the output; the host sums the
4 partials per batch and adds the output bias.

On-device dataflow (per core):
  phase A: PE-transpose x chunks (f32r, 1.5 cyc/row) into [f, t] layout,
           evicting PSUM in groups of 4 tiles alternately on DVE and ScalarE;
           project to QT/KT [dq, t] (feature-major) and V [t, dv]
           (token-major). V gets a 64-wide block of ones columns appended so
           the attn.V matmul also produces the softmax denominator
           replicated on psum partitions 64..127.
  phase B: per (q-chunk, head): scoresT[k, q] = KT_h.T @ QT_h on PE (f32r),
           exp(0.125 * s) on ScalarE in [128, 1024]-wide ACTIVATEs (scores
           are small, so no max-subtraction is needed), then
           outT'[128, q] = sum_k V''_h.T @ P. Rows 64..127 are the softmax
           denominator; normalize rows 0..63 via reciprocal_approx_fast +
           multiply on DVE.
           After all 4 heads of a q-chunk: out-projection matmuls for those
           4 token tiles (keeps PE fed while ACT runs exp for the next
           chunk).
"""

import ml_dtypes
import numpy as np

import concourse.bacc as bacc
import concourse.bass as bass
import concourse.mybir as mybir
import concourse.tile as tile
from concourse.bass_interp import get_hw_module
from concourse.bass_utils import run_bass_kernel_spmd
from concourse.masks import make_identity

# problem constants (hardcoded; must match the reference)
B = 2
S = 2048
D = 1024
NH = 16
DH = 64
SCALE = DH ** -0.5

# sharding
N_CORES = 8
HG = 4                # heads per core
F = HG * DH           # 256 projected features per core
CH = 512              # token chunk
NCH = S // CH         # 4 chunks
P = 128
FT = D // P           # 8 feature tiles
MT = F // P           # 2 projected-feature tiles
KT = S // P           # 16 key-token tiles

f32 = mybir.dt.float32
f32r = mybir.dt.float32r
bf16 = mybir.dt.bfloat16
EXP = mybir.ActivationFunctionType.Exp


def _emit(ctx, nc, tc, aps):
    xq, xk, xv, wqT, wkT, wvT, woT, bq2, bk2, bv1, out = aps

    consts = ctx.enter_context(tc.tile_pool(name="consts", bufs=1))
    persist = ctx.enter_context(tc.tile_pool(name="persist", bufs=1))
    # weights / biases to SBUF (q/k/v projection weights are loaded
    # per-input inside phase A to save SBUF)
    wo_sb = consts.tile([P, MT, D], f32r)
    nc.scalar.dma_start(out=wo_sb, in_=woT.rearrange("(m p) e -> p m e", p=P))
    bq_sb = consts.tile([P, MT], f32)
    bk_sb = consts.tile([P, MT], f32)
    nc.scalar.dma_start(out=bq_sb, in_=bq2)
    nc.scalar.dma_start(out=bk_sb, in_=bk2)
    bv_sb = consts.tile([P, F], f32)
    nc.scalar.dma_start(out=bv_sb, in_=bv1.unsqueeze(0).to_broadcast((P, F)))

    # persistent activations
    QT_sb = persist.tile([P, MT, NCH, CH], f32r)   # [dq%128, dq//128, qc, q]
    # KT, zero-padded to full-K contraction: variant par holds head parity
    # par's 64 feature rows, zeros in the other 64. A scores matmul then uses
    # a full [128, 128] stationary operand (K=64 descriptors run at half PE
    # rate), with the zeros annihilating the other head's QT rows.
    KTz_sb = persist.tile([P, 2, MT, NCH, CH], f32r)
    # V'' layout: [k%128, k//128, h, dv | 64 ones columns]
    V_sb = persist.tile([P, KT, HG, P], f32r)
    ones_sb = consts.tile([P, 1], f32)
    nc.vector.memset(ones_sb, 1.0)
    nc.vector.tensor_copy(
        V_sb[:, :, :, DH:P], ones_sb.to_broadcast((P, KT, HG, P - DH))
    )
    zeros_sb = consts.tile([P, 1], f32)
    nc.vector.memset(zeros_sb, 0.0)
    nc.vector.tensor_copy(
        KTz_sb[DH:P, 0], zeros_sb[DH:P].to_broadcast((DH, MT, NCH, CH))
    )
    nc.vector.tensor_copy(
        KTz_sb[0:DH, 1], zeros_sb[0:DH].to_broadcast((DH, MT, NCH, CH))
    )

    identity = consts.tile([P, P], f32r)
    id_tmp = consts.tile([P, P], f32)
    make_identity(nc, id_tmp)
    nc.vector.tensor_copy(identity, id_tmp)

    n_evict = 0
    with tc.tile_pool(name="x_stage", bufs=2) as x_stage, \
         tc.tile_pool(name="w_pool", bufs=2) as w_pool, \
         tc.tile_pool(name="xT_pool", bufs=3) as xT_pool, \
         tc.tile_pool(name="ps_tr", bufs=4, space="PSUM") as ps_tr, \
         tc.tile_pool(name="ps_proj", bufs=3, space="PSUM") as ps_proj:
        # phase A: transpose + project, per (input, chunk). K and V first so
        # attention (which needs all of K/V but only Q's first chunk) can
        # begin while Q's later chunks still project.
        for which, (x_ap, wT_ap) in enumerate(
            ((xk, wkT), (xv, wvT), (xq, wqT))
        ):
            w_sb = w_pool.tile([P, FT, F], f32r, tag="w")
            nc.scalar.dma_start(
                out=w_sb, in_=wT_ap.rearrange("(ft p) m -> p ft m", p=P)
            )
            for c in range(NCH):
                xT = xT_pool.tile([P, FT, CH], f32r, tag="xT")
                xs4 = x_stage.tile([P, CH // P, D], f32r, tag="xs")
                nc.sync.dma_start(
                    out=xs4,
                    in_=x_ap[c * CH:(c + 1) * CH, :].rearrange(
                        "(t4 p) f -> p t4 f", p=P
                    ),
                )
                for t4 in range(CH // P):
                    xs = xs4[:, t4, :]
                    for fg in range(2):  # groups of 4 feature tiles
                        ps = ps_tr.tile([P, 4, P], f32r, tag="tr")
                        for j in range(4):
                            ft = fg * 4 + j
                            nc.tensor.transpose(
                                ps[:, j, :], xs[:, ft * P:(ft + 1) * P],
                                identity,
                            )
                        dst = xT.rearrange("p (fg j) t -> p fg j t", fg=2)
                        if n_evict % 2 == 0:
                            nc.vector.tensor_copy(
                                dst[:, fg, :, t4 * P:(t4 + 1) * P], ps
                            )
                        else:
                            nc.scalar.copy(
                                dst[:, fg, :, t4 * P:(t4 + 1) * P], ps
                            )
                        n_evict += 1
                if which != 1:  # Q / K: feature-major [dq, t]
                    is_q = which == 2
                    b_sb = bq_sb if is_q else bk_sb
                    for m in range(MT):
                        ps = ps_proj.tile([P, CH], f32, tag="proj")
                        for ft in range(FT):
                            nc.tensor.matmul(
                                ps,
                                w_sb[:, ft, m * P:(m + 1) * P],
                                xT[:, ft, :],
                                start=(ft == 0),
                                stop=(ft == FT - 1),
                            )
                        if is_q:
                            nc.vector.tensor_scalar_add(
                                QT_sb[:, m, c, :], ps, b_sb[:, m:m + 1]
                            )
                        else:
                            nc.vector.tensor_scalar_add(
                                KTz_sb[0:DH, 0, m, c, :], ps[0:DH, :],
                                b_sb[0:DH, m:m + 1],
                            )
                            nc.vector.tensor_scalar_add(
                                KTz_sb[DH:P, 1, m, c, :], ps[DH:P, :],
                                b_sb[DH:P, m:m + 1],
                            )
                else:  # V: token-major [t, dv]
                    for t4 in range(CH // P):
                        ps = ps_proj.tile([P, F], f32, tag="proj")
                        for ft in range(FT):
                            nc.tensor.matmul(
                                ps,
                                xT[:, ft, t4 * P:(t4 + 1) * P],
                                w_sb[:, ft, :],
                                start=(ft == 0),
                                stop=(ft == FT - 1),
                            )
                        kt = c * (CH // P) + t4
                        nc.vector.tensor_add(
                            V_sb[:, kt, :, 0:DH],
                            ps.rearrange("p (h d) -> p h d", h=HG),
                            bv_sb.rearrange("p (h d) -> p h d", h=HG),
                        )

    with tc.tile_pool(name="ps_s", bufs=2, space="PSUM") as ps_s, \
         tc.tile_pool(name="ps_o", bufs=2, space="PSUM") as ps_o, \
         tc.tile_pool(name="ps_out", bufs=2, space="PSUM") as ps_out, \
         tc.tile_pool(name="pt_pool", bufs=1) as pt_pool, \
         tc.tile_pool(name="ot_pool", bufs=1) as ot_pool, \
         tc.tile_pool(name="o_stage", bufs=3) as o_stage, \
         tc.tile_pool(name="rc_pool", bufs=1) as rc_pool:
        OT_sb = ot_pool.tile([P, MT, NCH, CH], f32r)
        # phase B: attention per (q-chunk, head), then the chunk's out-proj
        for qc in range(NCH):
            for h in range(HG):
                mh, p0 = divmod(h, 2)
                p0 *= DH
                PT = pt_pool.tile([P, KT, CH], f32r, tag="PT")
                par = h % 2
                for kg in range(KT // 2):  # 2 k-tiles share a psum group
                    ps = ps_s.tile([P, 2, CH], f32, tag="s")
                    for j in range(2):
                        kt = kg * 2 + j
                        nc.tensor.matmul(
                            ps[:, j, :],
                            KTz_sb[:, par, mh, kt // 4,
                                   (kt % 4) * P:(kt % 4) * P + P],
                            QT_sb[:, mh, qc, :],
                            start=True,
                            stop=True,
                        )
                    nc.scalar.activation(
                        out=PT[:, kg * 2:kg * 2 + 2, :], in_=ps,
                        func=EXP, scale=SCALE,
                    )
                po = ps_o.tile([P, CH], f32, tag="o")
                for kt in range(KT):
                    nc.tensor.matmul(
                        po,
                        V_sb[:, kt, h, :],
                        PT[:, kt, :],
                        start=(kt == 0),
                        stop=(kt == KT - 1),
                    )
                rs = rc_pool.tile([DH, CH], f32, tag="rs")
                rc = rc_pool.tile([DH, CH], f32, tag="rc")
                nc.vector.tensor_copy(rs, po[DH:P, :])
                nc.vector.reciprocal_approx_fast(rc, rs)
                nc.vector.tensor_mul(
                    OT_sb[p0:p0 + DH, mh, qc, :], po[0:DH, :], rc
                )
            # out projection for this chunk's 4 token tiles
            for t4 in range(NCH):
                ob = o_stage.tile([P, D], f32, tag="ob")
                for n2 in range(D // CH):
                    ps = ps_out.tile([P, CH], f32, tag="po")
                    for m in range(MT):
                        nc.tensor.matmul(
                            ps,
                            OT_sb[:, m, qc, t4 * P:(t4 + 1) * P],
                            wo_sb[:, m, n2 * CH:(n2 + 1) * CH],
                            start=(m == 0),
                            stop=(m == MT - 1),
                        )
                    nc.vector.tensor_copy(ob[:, n2 * CH:(n2 + 1) * CH], ps)
                tt = qc * NCH + t4
                nc.sync.dma_start(out=out[tt * P:(tt + 1) * P, :], in_=ob)


def _build():
    nc = bacc.Bacc("TRN2", target_bir_lowering=False, debug=False)
    xq = nc.dram_tensor("xq", [S, D], f32r, kind="ExternalInput").ap()
    xk = nc.dram_tensor("xk", [S, D], f32r, kind="ExternalInput").ap()
    xv = nc.dram_tensor("xv", [S, D], f32r, kind="ExternalInput").ap()
    wqT = nc.dram_tensor("wqT", [D, F], f32r, kind="ExternalInput").ap()
    wkT = nc.dram_tensor("wkT", [D, F], f32r, kind="ExternalInput").ap()
    wvT = nc.dram_tensor("wvT", [D, F], f32r, kind="ExternalInput").ap()
    woT = nc.dram_tensor("woT", [F, D], f32r, kind="ExternalInput").ap()
    bq2 = nc.dram_tensor("bq2", [P, MT], f32, kind="ExternalInput").ap()
    bk2 = nc.dram_tensor("bk2", [P, MT], f32, kind="ExternalInput").ap()
    bv1 = nc.dram_tensor("bv1", [F], f32, kind="ExternalInput").ap()
    out = nc.dram_tensor("out", [S, D], f32, kind="ExternalOutput").ap()
    from contextlib import ExitStack

    with tile.TileContext(nc) as tc, ExitStack() as ctx:
        _emit(ctx, nc, tc,
              (xq, xk, xv, wqT, wkT, wvT, woT, bq2, bk2, bv1, out))
    nc.compile()
    nc.m = get_hw_module(nc.m)
    return nc


_cached_nc = None


def _get_nc():
    global _cached_nc
    if _cached_nc is None:
        _cached_nc = _build()
    return _cached_nc


def make_in_maps(query, key, value, Wq, bq, Wk, bk, Wv, bv, Wo, bo):
    query, key, value, Wq, bq, Wk, bk, Wv, bv, Wo = (
        np.asarray(a, np.float32)
        for a in (query, key, value, Wq, bq, Wk, bk, Wv, bv, Wo)
    )
    in_maps = []
    for c in range(N_CORES):
        b, g = divmod(c, 4)
        fs = slice(g * F, (g + 1) * F)
        in_maps.append({
            "xq": np.ascontiguousarray(query[b]),
            "xk": np.ascontiguousarray(key[b]),
            "xv": np.ascontiguousarray(value[b]),
            "wqT": np.ascontiguousarray(Wq[fs].T),
            "wkT": np.ascontiguousarray(Wk[fs].T),
            "wvT": np.ascontiguousarray(Wv[fs].T),
            "woT": np.ascontiguousarray(Wo[:, fs].T),
            "bq2": np.ascontiguousarray(bq[fs].reshape(MT, P).T),
            "bk2": np.ascontiguousarray(bk[fs].reshape(MT, P).T),
            "bv1": np.ascontiguousarray(bv[fs]),
        })
    return in_maps


def combine_outputs(core_outs, bo):
    bo = np.asarray(bo, np.float32)
    out = np.empty((B, S, D), np.float32)
    for b in range(B):
        acc = core_outs[4 * b].astype(np.float32)
        for g in range(1, 4):
            acc = acc + core_outs[4 * b + g]
        out[b] = acc + bo
    return out


def kernel(query, key, value, Wq, bq, Wk, bk, Wv, bv, Wo, bo, **run_kwargs):
    nc = _get_nc()
    in_maps = make_in_maps(query, key, value, Wq, bq, Wk, bk, Wv, bv, Wo, bo)
    res = run_bass_kernel_spmd(
        nc, in_maps, core_ids=list(range(N_CORES)), **run_kwargs
    )
    out = combine_outputs([r["out"] for r in res.results], bo)
    if run_kwargs:
        kernel.last_results = res
    return out
